# revision 1
# baseline (speedup 1.0000x reference)
"""Multi-head causal attention (B=2, S=2048, D=1024, H=16) on 8 TRN2 NeuronCores.

Sharding: batch*head parallel. Core c handles batch b = c//4 and the 4
heads h in [4*(c%4), 4*(c%4)+4). Each core computes its heads' Q/K/V
projections (column-parallel), causal softmax attention, and its partial
row-parallel output projection; the host sums the 4 partial outputs per
batch (the AllReduce of row-parallel tensor parallelism).

On-device layout: everything is kept "transposed" (feature-major) so
every matmul contracts along the partition dimension:
  scoresT[k,q] = K Q^T      (per head, 128-row k-tiles x 512-col q-tiles)
  P^T = exp(scoresT/8 + mask/8)   (additive -1e9 causal mask, PE-accumulated)
  outT[d,q]   = sum_k V[k,d] P^T[k,q]   (PSUM-accumulated over k-tiles)
  sums[q]     = sum_k P^T[k,q]          (ones-vector matmul, col-packed)
  y[q,e]     += sum_hd outT_norm[hd,q] * w_oT[hd,e]
Softmax skips the max-subtraction: scores ~ N(0,1), so exp never
overflows fp32, and exp(-1e9/8) underflows to exactly 0 like the
reference's masked_fill(-1e9).

Matmuls run as float32r (TF32-like, 1 cycle/row at N>=512; measured
~1.5e-4 rms per matmul). Fully-masked 128x512 blocks are skipped
(causal => ~62% of blocks computed).
"""

import numpy as np

D_MODEL = 1024
N_HEADS = 16
D_K = 64
B, S = 2, 2048
N_CORES = 8
HPC = 4            # heads per core
KT = S // 128      # 16 k-tiles
QT = S // 512      # 4 q-tiles
ET = D_MODEL // 128  # 8 e-tiles (contraction tiles for projections)

ATT_BF16 = False  # bf16 scores/attnV matmuls (f32r projections + output proj)
ET_BF16 = True   # bf16 exp output + V operand for the attnV matmul only

_PROG_CACHE = {}


def _build_program():
    import concourse.bacc as bacc_mod
    import concourse.mybir as mybir
    import concourse.tile as tile

    f32 = mybir.dt.float32
    f32r = mybir.dt.float32r
    bf16 = mybir.dt.bfloat16
    att_dt = bf16 if ATT_BF16 else f32r
    et_dt = bf16 if (ATT_BF16 or ET_BF16) else f32r
    Exp = mybir.ActivationFunctionType.Exp

    nc = bacc_mod.Bacc(
        "TRN2", target_bir_lowering=False, debug=False, num_devices=N_CORES
    )

    xq = nc.dram_tensor("xq", [D_MODEL, S], f32r, kind="ExternalInput").ap()
    xk = nc.dram_tensor("xk", [D_MODEL, S], f32r, kind="ExternalInput").ap()
    xv = nc.dram_tensor("xv", [D_MODEL, S], f32r, kind="ExternalInput").ap()
    wq = nc.dram_tensor("wq", [D_MODEL, 256], f32r, kind="ExternalInput").ap()
    wk = nc.dram_tensor("wk", [D_MODEL, 256], f32r, kind="ExternalInput").ap()
    wv = nc.dram_tensor("wv", [D_MODEL, 256], f32r, kind="ExternalInput").ap()
    wo = nc.dram_tensor("wo", [256, D_MODEL], f32r, kind="ExternalInput").ap()
    maskt = nc.dram_tensor("maskt", [128, 2048], mybir.dt.bfloat16, kind="ExternalInput").ap()
    idbf = nc.dram_tensor("idbf", [128, 132], mybir.dt.bfloat16, kind="ExternalInput").ap()
    consts = nc.dram_tensor("consts", [128, 193], f32r, kind="ExternalInput").ap()
    y = nc.dram_tensor("y", [S, D_MODEL], f32, kind="ExternalOutput").ap()

    with (
        tile.TileContext(nc) as tc,
        nc.allow_low_precision("fp32r attention"),
        tc.tile_pool(name="persist", bufs=1) as pp,
    ):
        # ---- persistent SBUF tiles ----
        def persist(shape, dtype, name):
            return pp.tile(shape, dtype, name=name, tag=name)

        wq_sb = persist([128, ET * 256], f32r, "wq_sb")
        wk_sb = persist([128, ET * 256], f32r, "wk_sb")
        wv_sb = persist([128, ET * 256], f32r, "wv_sb")
        wo_sb = [persist([128, D_MODEL], f32r, f"wo_sb{p}") for p in range(2)]
        maskt_sb = persist([128, 2048], mybir.dt.bfloat16, "maskt_sb")
        idbf_sb = persist([128, 132], mybir.dt.bfloat16, "idbf_sb")
        consts_sb = persist([128, 193], f32r, "consts_sb")
        qt_sb = [persist([128, S], att_dt, f"qt_sb{p}") for p in range(2)]
        kt_sb = [persist([128, S], att_dt, f"kt_sb{p}") for p in range(2)]
        v_sb = [persist([128, 260], et_dt, f"v_sb{i}") for i in range(KT)]
        outt_sb = [persist([128, S], f32r, f"outt_sb{p}") for p in range(2)]

        identity = consts_sb[:, 0:128]
        ones_col = consts_sb[:, 128:192]   # [128, 64] of 1.0
        ones1 = consts_sb[:, 192:193]      # [128, 1] of 1.0

        # small consts first (the PE warm-up pack depends on them)
        nc.sync.dma_start(out=consts_sb[:], in_=consts[:])
        nc.sync.dma_start(out=idbf_sb[:], in_=idbf[:])
        nc.sync.dma_start(out=maskt_sb[:], in_=maskt[:])
        # weight loads: [1024, 256] -> [128, 8*256] (e-tile t at cols 256t)
        for w_dram, w_tile in ((wq, wq_sb), (wk, wk_sb), (wv, wv_sb)):
            nc.sync.dma_start(
                out=w_tile[:].rearrange("p (t d) -> p t d", t=ET),
                in_=w_dram.rearrange("(t p) d -> p t d", p=128),
            )
        for p in range(2):
            nc.sync.dma_start(out=wo_sb[p][:], in_=wo[p * 128 : (p + 1) * 128, :])

        # ---- PE warm-up ----
        # The PE HAM clock gate starts (and re-enters) K=4/8 half-clock and
        # only returns to full clock after ~3.4us of gapless PE activity.
        # Dense same-stationary dummy matmuls (results never read) force the
        # transition; packs are re-issued wherever the schedule has an
        # unavoidable multi-us PE idle (DMA-bound ramp, phase boundaries,
        # softmax-normalize tails).
        def emit_warm_pack(pool, count, tag="warm", name="warm"):
            wt = pool.tile([128, 512], f32, name=name, tag=tag)
            for w in range(count):
                nc.tensor.matmul(
                    wt[:],
                    idbf_sb[:, 0:128],
                    maskt_sb[:, 0:512],
                    start=True,
                    stop=True,
                )

        with tc.tile_pool(name="psW", bufs=1, space="PSUM") as psW:
            emit_warm_pack(psW, 24, name="warm_start")

        # ---- phase B: projections ----
        # Q^T/K^T accumulate over all 8 e-tiles into [128, 2048] PSUM (8
        # banks, both m-tiles). The strided xv DMAs are emitted interleaved
        # with the xq/xk streams so the V-projection (which must wait for
        # the QK PSUM banks anyway) starts with its data already resident
        # and runs as a dense PE burst instead of being DMA-paced.
        with (
            tc.tile_pool(name="xe", bufs=3) as xep,
            tc.tile_pool(name="xvk", bufs=10) as xvkp,
        ):
            vdma_tiles = []

            def emit_v_dma():
                i = len(vdma_tiles)
                xvk = xvkp.tile([128, ET * 128], f32r, name=f"xvk_{i}", tag="xvk")
                nc.sync.dma_start(
                    out=xvk[:].rearrange("p (t k) -> p t k", t=ET),
                    in_=xv[:, i * 128 : (i + 1) * 128].rearrange(
                        "(t p) k -> p t k", p=128
                    ),
                )
                vdma_tiles.append(xvk)

            psA_ctx = tc.tile_pool(name="psA", bufs=1, space="PSUM")
            psA = psA_ctx.__enter__()
            for ti, (x_dram, w_tile, dst) in enumerate(
                ((xq, wq_sb, qt_sb), (xk, wk_sb, kt_sb))
            ):
                ps = [
                    psA.tile(
                        [128, S], f32, name=f"ps_p{ti}_{m}", tag=f"proj{m}", bufs=1
                    )
                    for m in range(2)
                ]
                for e in range(ET):
                    xe = xep.tile([128, S], f32r, name=f"xe_{ti}_{e}", tag="xe")
                    nc.sync.dma_start(out=xe[:], in_=x_dram[e * 128 : (e + 1) * 128, :])
                    if ti == 1 or e >= 6:
                        emit_v_dma()
                    for m in range(2):
                        lhsT = w_tile[:, e * 256 + m * 128 : e * 256 + (m + 1) * 128]
                        for n in range(QT):
                            nc.tensor.matmul(
                                ps[m][:, n * 512 : (n + 1) * 512],
                                lhsT,
                                xe[:, n * 512 : (n + 1) * 512],
                                start=(e == 0),
                                stop=(e == ET - 1),
                            )
                for m in range(2):
                    nc.vector.tensor_copy(dst[m][:], ps[m][:])

            psA_ctx.__exit__(None, None, None)
            psV_ctx = tc.tile_pool(name="psV", bufs=2, space="PSUM")
            psV = psV_ctx.__enter__()
            # V projection: dense burst (data already largely resident)
            for i in range(KT):
                if i >= len(vdma_tiles) - 2 and len(vdma_tiles) < KT:
                    emit_v_dma()
                psv = psV.tile([128, 256], f32, name=f"psv_{i}", tag="v")
                xvk = vdma_tiles[i]
                for e in range(ET):
                    nc.tensor.matmul(
                        psv[:],
                        xvk[:, e * 128 : (e + 1) * 128],
                        wv_sb[:, e * 256 : (e + 1) * 256],
                        start=(e == 0),
                        stop=(e == ET - 1),
                    )
                nc.vector.tensor_copy(
                    v_sb[i][:].rearrange("p (h c) -> p h c", c=65)[:, :, 0:64],
                    psv[:].rearrange("p (h d) -> p h d", d=64),
                )
                ones4 = idbf_sb[:, 128:132] if (ATT_BF16 or ET_BF16) else consts_sb[:, 128:132]
                nc.vector.tensor_copy(
                    v_sb[i][:].rearrange("p (h c) -> p h c", c=65)[:, :, 64:65],
                    ones4.rearrange("p (h c) -> p h c", c=1),
                )
            while len(vdma_tiles) < KT:
                emit_v_dma()
            psV_ctx.__exit__(None, None, None)

        # ---- phase C+D: attention with interleaved output projection ----
        # One head-pair per pass (pr = 0, 1). Per (pr, j): score tiles are
        # [128, 1024] head-pair PSUM tiles (row-packed score MMs fill the two
        # banks concurrently; ONE exp per round at FD=1024 runs ~2x faster
        # per element). attnV accumulates into a [65, 1024] pair tile (row
        # 64 = sum of exp via the ones column of v_sb). Normalization of
        # q-block j-1 is emitted lazily inside block j so its DVE chain and
        # broadcast matmuls never stall the PE; the output projection of
        # block j-1 runs as dense filler inside the pr=1 pass.
        with (
            tc.tile_pool(name="psS", bufs=3, space="PSUM") as psS,
            tc.tile_pool(name="psO", bufs=1, space="PSUM") as psO,
            tc.tile_pool(name="et", bufs=6) as etp,
            tc.tile_pool(name="bcsb", bufs=3) as bcp,
            tc.tile_pool(name="rcsb", bufs=3) as rcp,
            tc.tile_pool(name="ysb", bufs=3) as ysbp,
        ):
            def emit_outproj_mtile(m):
                psy = psS.tile([128, 1024], f32, name=f"psy_{m}", tag="s")
                for p in range(2):
                    for n in range(2):
                        nc.tensor.matmul(
                            psy[:, n * 512 : (n + 1) * 512],
                            outt_sb[p][:, m * 128 : (m + 1) * 128],
                            wo_sb[p][:, n * 512 : (n + 1) * 512],
                            start=(p == 0),
                            stop=(p == 1),
                        )
                y_sb = ysbp.tile([128, 1024], f32, name=f"y_sb_{m}", tag="ysb")
                nc.vector.tensor_copy(y_sb[:], psy[:])
                nc.sync.dma_start(out=y[m * 128 : (m + 1) * 128, :], in_=y_sb[:])

            def emit_normalize(pr, jj, ps_out_prev):
                qsj = slice(jj * 512, (jj + 1) * 512)
                ssb = rcp.tile([33, 512], f32, name=f"ssb_{pr}_{jj}", tag="ssb")
                for hh in range(2):
                    nc.vector.tensor_copy(
                        ssb[32 * hh : 32 * hh + 1, :],
                        ps_out_prev[64:65, 512 * hh : 512 * (hh + 1)],
                    )
                rc32 = rcp.tile([33, 512], f32, name=f"rc32_{pr}_{jj}", tag="rc32")
                nc.vector.reciprocal_approx_fast(out=rc32[:], in_=ssb[:])
                rc = rcp.tile([33, 512], f32r, name=f"rc_{pr}_{jj}", tag="rc")
                nc.vector.tensor_copy(rc[:], rc32[:])
                bc = psS.tile([128, 1024], f32, name=f"ps_bc_{pr}_{jj}", tag="s")
                for hh in range(2):
                    nc.tensor.matmul(
                        bc[0:64, 512 * hh : 512 * (hh + 1)],
                        consts_sb[32 * hh : 32 * hh + 1, 128:192],
                        rc[32 * hh : 32 * hh + 1, :],
                        start=True,
                        stop=True,
                        tile_position=(32 * hh, 0),
                    )
                bc_sb = bcp.tile([64, 1024], f32, name=f"bc_sb_{pr}_{jj}", tag="bc")
                nc.vector.tensor_copy(bc_sb[:], bc[0:64, :])
                for hh in range(2):
                    nc.vector.tensor_mul(
                        outt_sb[pr][64 * hh : 64 * hh + 64, qsj],
                        ps_out_prev[0:64, 512 * hh : 512 * (hh + 1)],
                        bc_sb[:, 512 * hh : 512 * (hh + 1)],
                    )

            for pr in range(2):
                pending_norm = None  # (pr, j, ps_out) awaiting lazy normalize
                pending_out = None   # q-block awaiting output projection (pr=1)
                j_order = range(QT) if pr == 0 else range(QT - 1, -1, -1)
                for j in j_order:
                    n_i = 4 * j + 4
                    qs = slice(j * 512, (j + 1) * 512)
                    ps_out = psO.tile(
                        [65, 1024], f32, name=f"ps_out_{pr}_{j}", tag="o"
                    )
                    prev_et = None
                    prev_i = -1
                    for i in range(n_i):
                        diag = i >= 4 * j
                        r = i - 4 * j
                        pss = psS.tile(
                            [128, 1024], f32, name=f"ps_s{pr}_{j}_{i}", tag="s"
                        )
                        if diag:
                            nw = 128 * (r + 1)
                            for hh in range(2):
                                nc.tensor.matmul(
                                    pss[:, 512 * hh : 512 * hh + nw],
                                    idbf_sb[:, 0:128],
                                    maskt_sb[:, r * 512 : r * 512 + nw],
                                    start=True,
                                    stop=False,
                                )
                        for hh in range(2):
                            hp = slice(64 * hh, 64 * hh + 64)
                            nc.tensor.matmul(
                                pss[:, 512 * hh : 512 * (hh + 1)],
                                kt_sb[pr][hp, i * 128 : (i + 1) * 128],
                                qt_sb[pr][hp, qs],
                                start=not diag,
                                stop=True,
                            )
                        et = etp.tile(
                            [128, 1024], et_dt, name=f"et{pr}_{j}_{i}", tag="et"
                        )
                        nc.scalar.activation(et[:], pss[:], Exp, scale=0.125)
                        if prev_et is not None:
                            for hh in range(2):
                                nc.tensor.matmul(
                                    ps_out[:, 512 * hh : 512 * (hh + 1)],
                                    v_sb[prev_i][:, (2 * pr + hh) * 65 : (2 * pr + hh + 1) * 65],
                                    prev_et[:, 512 * hh : 512 * (hh + 1)],
                                    start=(prev_i == 0),
                                    stop=(prev_i == n_i - 1),
                                )
                        prev_et, prev_i = et, i
                        if i == 1 and pending_norm is not None:
                            pn_j = pending_norm[1]
                            emit_normalize(*pending_norm)
                            pending_norm = None
                            if pr == 1:
                                pending_out = pn_j
                        if pending_out is not None and i == 2:
                            for m in range(4 * pending_out, 4 * pending_out + 4):
                                emit_outproj_mtile(m)
                            pending_out = None
                    for hh in range(2):
                        nc.tensor.matmul(
                            ps_out[:, 512 * hh : 512 * (hh + 1)],
                            v_sb[n_i - 1][:, (2 * pr + hh) * 65 : (2 * pr + hh + 1) * 65],
                            prev_et[:, 512 * hh : 512 * (hh + 1)],
                            start=(n_i - 1 == 0),
                            stop=True,
                        )
                    pending_norm = (pr, j, ps_out)
                emit_normalize(*pending_norm)
                if pr == 1:
                    if pending_out is not None:
                        for m in range(4 * pending_out, 4 * pending_out + 4):
                            emit_outproj_mtile(m)
                    # pr1 runs j descending, so the pass ends on j=0
                    for m in range(0, 4):
                        emit_outproj_mtile(m)

    nc.compile()
    return nc


def _get_program():
    if "nc" not in _PROG_CACHE:
        _PROG_CACHE["nc"] = _build_program()
    return _PROG_CACHE["nc"]


def _host_prep(query, key, value, mask, w_q, w_k, w_v, w_o):
    query = np.asarray(query, dtype=np.float32)
    key = np.asarray(key, dtype=np.float32)
    value = np.asarray(value, dtype=np.float32)
    w_q = np.asarray(w_q, dtype=np.float32)
    w_k = np.asarray(w_k, dtype=np.float32)
    w_v = np.asarray(w_v, dtype=np.float32)
    w_o = np.asarray(w_o, dtype=np.float32)
    m = np.asarray(mask).reshape(S, S).astype(bool)

    # The kernel's block-skip structure assumes the standard causal mask.
    expected = np.triu(np.ones((S, S), dtype=bool), k=1)
    if not np.array_equal(m, expected):
        raise NotImplementedError("kernel specialized for causal (triu, k=1) mask")

    # 4 canonical diagonal-straddle mask tiles: pattern r covers k-tile
    # 4j+r vs q-tile j; masked where (128r + row) > col.
    import ml_dtypes

    maskt = np.zeros((128, 2048), dtype=np.float32)
    rows = np.arange(128)[:, None]
    cols = np.arange(512)[None, :]
    for r in range(4):
        maskt[:, r * 512 : (r + 1) * 512] = np.where(
            (128 * r + rows) > cols, np.float32(-1e9), np.float32(0.0)
        )
    maskt = maskt.astype(ml_dtypes.bfloat16)
    idbf = np.zeros((128, 132), dtype=ml_dtypes.bfloat16)
    idbf[:, 0:128] = np.eye(128, dtype=ml_dtypes.bfloat16)
    idbf[:, 128:132] = ml_dtypes.bfloat16(1.0)

    consts = np.zeros((128, 193), dtype=np.float32)
    consts[:, 0:128] = np.eye(128, dtype=np.float32)
    consts[:, 128:193] = 1.0

    xt = {}
    for b in range(B):
        xt[("q", b)] = np.ascontiguousarray(query[b].T)
        xt[("k", b)] = np.ascontiguousarray(key[b].T)
        xt[("v", b)] = np.ascontiguousarray(value[b].T)

    in_maps = []
    for c in range(N_CORES):
        b = c // 4
        hb = (c % 4) * HPC
        rs = slice(hb * D_K, (hb + HPC) * D_K)
        in_maps.append(
            {
                "xq": xt[("q", b)],
                "xk": xt[("k", b)],
                "xv": xt[("v", b)],
                "wq": np.ascontiguousarray(w_q[rs, :].T),
                "wk": np.ascontiguousarray(w_k[rs, :].T),
                "wv": np.ascontiguousarray(w_v[rs, :].T),
                "wo": np.ascontiguousarray(w_o[:, rs].T),
                "maskt": maskt,
                "idbf": idbf,
                "consts": consts,
            }
        )
    return in_maps


def kernel(query, key, value, mask, w_q, w_k, w_v, w_o):
    from concourse.bass_utils import run_bass_kernel_spmd

    in_maps = _host_prep(query, key, value, mask, w_q, w_k, w_v, w_o)
    nc = _get_program()
    res = run_bass_kernel_spmd(nc, in_maps, list(range(N_CORES)))
    out = np.zeros((B, S, D_MODEL), dtype=np.float32)
    for c in range(N_CORES):
        out[c // 4] += res.results[c]["y"]
    return out



# revision 9
# speedup vs baseline: 1.1781x; 1.1781x over previous
"""Multi-head causal attention (B=2, S=2048, D=1024, H=16) on 8 TRN2 NeuronCores.

Sharding: batch*head parallel. Core c handles batch b = c//4 and the 4
heads h in [4*(c%4), 4*(c%4)+4). Each core computes its heads' Q/K/V
projections (column-parallel), causal softmax attention, and its partial
row-parallel output projection; the host sums the 4 partial outputs per
batch (the AllReduce of row-parallel tensor parallelism).

On-device layout: everything is kept "transposed" (feature-major) so
every matmul contracts along the partition dimension:
  scoresT[k,q] = K Q^T      (per head, 128-row k-tiles x 512-col q-tiles)
  P^T = exp(scoresT/8 + mask/8)   (additive -1e9 causal mask, PE-accumulated)
  outT[d,q]   = sum_k V[k,d] P^T[k,q]   (PSUM-accumulated over k-tiles)
  sums[q]     = sum_k P^T[k,q]          (ones-vector matmul, col-packed)
  y[q,e]     += sum_hd outT_norm[hd,q] * w_oT[hd,e]
Softmax skips the max-subtraction: scores ~ N(0,1), so exp never
overflows fp32, and exp(-1e9/8) underflows to exactly 0 like the
reference's masked_fill(-1e9).

Matmuls run as float32r (TF32-like, 1 cycle/row at N>=512; measured
~1.5e-4 rms per matmul). Fully-masked 128x512 blocks are skipped
(causal => ~62% of blocks computed).
"""

import numpy as np

D_MODEL = 1024
N_HEADS = 16
D_K = 64
B, S = 2, 2048
N_CORES = 8
HPC = 4            # heads per core
KT = S // 128      # 16 k-tiles
QT = S // 512      # 4 q-tiles
ET = D_MODEL // 128  # 8 e-tiles (contraction tiles for projections)

ATT_BF16 = True  # bf16 scores/attnV matmuls (f32r output proj)
ET_BF16 = True   # bf16 exp output + V operand for the attnV matmul only
X_BF16 = True    # bf16 x inputs + q/k/v weights (halves input DMA)
Y_BF16 = True    # bf16 y output (halves output DMA; host upcasts)

_PROG_CACHE = {}


def _build_program():
    import concourse.bacc as bacc_mod
    import concourse.mybir as mybir
    import concourse.tile as tile

    f32 = mybir.dt.float32
    f32r = mybir.dt.float32r
    bf16 = mybir.dt.bfloat16
    att_dt = bf16 if ATT_BF16 else f32r
    et_dt = bf16 if (ATT_BF16 or ET_BF16) else f32r
    x_dt = bf16 if X_BF16 else f32r
    y_dt = bf16 if Y_BF16 else f32
    Exp = mybir.ActivationFunctionType.Exp

    nc = bacc_mod.Bacc(
        "TRN2", target_bir_lowering=False, debug=False, num_devices=N_CORES
    )

    xq = nc.dram_tensor("xq", [D_MODEL, S], x_dt, kind="ExternalInput").ap()
    xk = nc.dram_tensor("xk", [D_MODEL, S], x_dt, kind="ExternalInput").ap()
    xv = nc.dram_tensor("xv", [D_MODEL, S], x_dt, kind="ExternalInput").ap()
    wq = nc.dram_tensor("wq", [D_MODEL, 256], x_dt, kind="ExternalInput").ap()
    wk = nc.dram_tensor("wk", [D_MODEL, 256], x_dt, kind="ExternalInput").ap()
    wv = nc.dram_tensor("wv", [D_MODEL, 256], x_dt, kind="ExternalInput").ap()
    wo = nc.dram_tensor("wo", [256, D_MODEL], f32r, kind="ExternalInput").ap()
    maskt = nc.dram_tensor("maskt", [128, 2048], mybir.dt.bfloat16, kind="ExternalInput").ap()
    idbf = nc.dram_tensor("idbf", [128, 132], mybir.dt.bfloat16, kind="ExternalInput").ap()
    consts = nc.dram_tensor("consts", [128, 193], f32r, kind="ExternalInput").ap()
    y = nc.dram_tensor("y", [S, D_MODEL], y_dt, kind="ExternalOutput").ap()

    with (
        tile.TileContext(nc) as tc,
        nc.allow_low_precision("fp32r attention"),
        tc.tile_pool(name="persist", bufs=1) as pp,
    ):
        # ---- persistent SBUF tiles ----
        def persist(shape, dtype, name):
            return pp.tile(shape, dtype, name=name, tag=name)

        wq_sb = persist([128, ET * 256], x_dt, "wq_sb")
        wk_sb = persist([128, ET * 256], x_dt, "wk_sb")
        wv_sb = persist([128, ET * 256], x_dt, "wv_sb")
        wo_sb = [persist([128, D_MODEL], f32r, f"wo_sb{p}") for p in range(2)]
        maskt_sb = persist([128, 2048], mybir.dt.bfloat16, "maskt_sb")
        idbf_sb = persist([128, 132], mybir.dt.bfloat16, "idbf_sb")
        consts_sb = persist([128, 193], f32r, "consts_sb")
        qt_sb = [persist([128, S], att_dt, f"qt_sb{p}") for p in range(2)]
        kt_sb = [persist([128, S], att_dt, f"kt_sb{p}") for p in range(2)]
        v_sb = [persist([128, 260], et_dt, f"v_sb{i}") for i in range(KT)]
        outt_sb = [persist([128, S], f32r, f"outt_sb{p}") for p in range(2)]

        identity = consts_sb[:, 0:128]
        ones_col = consts_sb[:, 128:192]   # [128, 64] of 1.0
        ones1 = consts_sb[:, 192:193]      # [128, 1] of 1.0

        # small consts first (the PE warm-up pack depends on them)
        nc.sync.dma_start(out=consts_sb[:], in_=consts[:])
        nc.sync.dma_start(out=idbf_sb[:], in_=idbf[:])
        nc.sync.dma_start(out=maskt_sb[:], in_=maskt[:])
        # weight loads: [1024, 256] -> [128, 8*256] (e-tile t at cols 256t)
        for w_dram, w_tile in ((wq, wq_sb), (wk, wk_sb), (wv, wv_sb)):
            nc.sync.dma_start(
                out=w_tile[:].rearrange("p (t d) -> p t d", t=ET),
                in_=w_dram.rearrange("(t p) d -> p t d", p=128),
            )
        for p in range(2):
            nc.sync.dma_start(out=wo_sb[p][:], in_=wo[p * 128 : (p + 1) * 128, :])

        # ---- PE warm-up ----
        # The PE HAM clock gate starts (and re-enters) K=4/8 half-clock and
        # only returns to full clock after ~3.4us of gapless PE activity.
        # Dense same-stationary dummy matmuls (results never read) force the
        # transition; packs are re-issued wherever the schedule has an
        # unavoidable multi-us PE idle (DMA-bound ramp, phase boundaries,
        # softmax-normalize tails).
        def emit_warm_pack(pool, count, tag="warm", name="warm"):
            wt = pool.tile([128, 512], f32, name=name, tag=tag)
            for w in range(count):
                nc.tensor.matmul(
                    wt[:],
                    idbf_sb[:, 0:128],
                    maskt_sb[:, 0:512],
                    start=True,
                    stop=True,
                )

        with tc.tile_pool(name="psW", bufs=1, space="PSUM") as psW:
            emit_warm_pack(psW, 24, name="warm_start")

        # ---- phase B: projections ----
        # Q^T/K^T accumulate over all 8 e-tiles into [128, 2048] PSUM (8
        # banks, both m-tiles). The strided xv DMAs are emitted interleaved
        # with the xq/xk streams so the V-projection (which must wait for
        # the QK PSUM banks anyway) starts with its data already resident
        # and runs as a dense PE burst instead of being DMA-paced.
        with (
            tc.tile_pool(name="xe", bufs=3) as xep,
            tc.tile_pool(name="xvk", bufs=10) as xvkp,
        ):
            vdma_tiles = []

            def emit_v_dma():
                i = len(vdma_tiles)
                xvk = xvkp.tile([128, ET * 128], x_dt, name=f"xvk_{i}", tag="xvk")
                nc.sync.dma_start(
                    out=xvk[:].rearrange("p (t k) -> p t k", t=ET),
                    in_=xv[:, i * 128 : (i + 1) * 128].rearrange(
                        "(t p) k -> p t k", p=128
                    ),
                )
                vdma_tiles.append(xvk)

            psA_ctx = tc.tile_pool(name="psA", bufs=1, space="PSUM")
            psA = psA_ctx.__enter__()
            for ti, (x_dram, w_tile, dst) in enumerate(
                ((xq, wq_sb, qt_sb), (xk, wk_sb, kt_sb))
            ):
                ps = [
                    psA.tile(
                        [128, S], f32, name=f"ps_p{ti}_{m}", tag=f"proj{m}", bufs=1
                    )
                    for m in range(2)
                ]
                for e in range(ET):
                    xe = xep.tile([128, S], x_dt, name=f"xe_{ti}_{e}", tag="xe")
                    nc.sync.dma_start(out=xe[:], in_=x_dram[e * 128 : (e + 1) * 128, :])
                    if ti == 1 or e >= 6:
                        emit_v_dma()
                    for m in range(2):
                        lhsT = w_tile[:, e * 256 + m * 128 : e * 256 + (m + 1) * 128]
                        for n in range(QT):
                            nc.tensor.matmul(
                                ps[m][:, n * 512 : (n + 1) * 512],
                                lhsT,
                                xe[:, n * 512 : (n + 1) * 512],
                                start=(e == 0),
                                stop=(e == ET - 1),
                            )
                for m in range(2):
                    nc.vector.tensor_copy(dst[m][:], ps[m][:])

            psA_ctx.__exit__(None, None, None)
            psV_ctx = tc.tile_pool(name="psV", bufs=2, space="PSUM")
            psV = psV_ctx.__enter__()
            # V projection: dense burst (data already largely resident)
            for i in range(KT):
                if i >= len(vdma_tiles) - 2 and len(vdma_tiles) < KT:
                    emit_v_dma()
                psv = psV.tile([128, 256], f32, name=f"psv_{i}", tag="v")
                xvk = vdma_tiles[i]
                for e in range(ET):
                    nc.tensor.matmul(
                        psv[:],
                        xvk[:, e * 128 : (e + 1) * 128],
                        wv_sb[:, e * 256 : (e + 1) * 256],
                        start=(e == 0),
                        stop=(e == ET - 1),
                    )
                nc.vector.tensor_copy(
                    v_sb[i][:].rearrange("p (h c) -> p h c", c=65)[:, :, 0:64],
                    psv[:].rearrange("p (h d) -> p h d", d=64),
                )
                ones4 = idbf_sb[:, 128:132] if (ATT_BF16 or ET_BF16) else consts_sb[:, 128:132]
                nc.vector.tensor_copy(
                    v_sb[i][:].rearrange("p (h c) -> p h c", c=65)[:, :, 64:65],
                    ones4.rearrange("p (h c) -> p h c", c=1),
                )
            while len(vdma_tiles) < KT:
                emit_v_dma()
            psV_ctx.__exit__(None, None, None)

        # ---- phase C+D: attention with interleaved output projection ----
        # One head-pair per pass (pr = 0, 1). Per (pr, j): score tiles are
        # [128, 1024] head-pair PSUM tiles (row-packed score MMs fill the two
        # banks concurrently; ONE exp per round at FD=1024 runs ~2x faster
        # per element). attnV accumulates into a [65, 1024] pair tile (row
        # 64 = sum of exp via the ones column of v_sb). Normalization of
        # q-block j-1 is emitted lazily inside block j so its DVE chain and
        # broadcast matmuls never stall the PE; the output projection of
        # block j-1 runs as dense filler inside the pr=1 pass.
        with (
            tc.tile_pool(name="psS", bufs=3, space="PSUM") as psS,
            tc.tile_pool(name="psO", bufs=1, space="PSUM") as psO,
            tc.tile_pool(name="et", bufs=6) as etp,
            tc.tile_pool(name="bcsb", bufs=3) as bcp,
            tc.tile_pool(name="rcsb", bufs=3) as rcp,
            tc.tile_pool(name="ysb", bufs=3) as ysbp,
        ):
            def emit_outproj_mtile(m):
                psy = psS.tile([128, 1024], f32, name=f"psy_{m}", tag="s")
                for p in range(2):
                    for n in range(2):
                        nc.tensor.matmul(
                            psy[:, n * 512 : (n + 1) * 512],
                            outt_sb[p][:, m * 128 : (m + 1) * 128],
                            wo_sb[p][:, n * 512 : (n + 1) * 512],
                            start=(p == 0),
                            stop=(p == 1),
                        )
                y_sb = ysbp.tile([128, 1024], y_dt, name=f"y_sb_{m}", tag="ysb")
                nc.vector.tensor_copy(y_sb[:], psy[:])
                nc.sync.dma_start(out=y[m * 128 : (m + 1) * 128, :], in_=y_sb[:])

            def emit_normalize(pr, jj, ps_out_prev):
                qsj = slice(jj * 512, (jj + 1) * 512)
                ssb = rcp.tile([33, 512], f32, name=f"ssb_{pr}_{jj}", tag="ssb")
                for hh in range(2):
                    nc.vector.tensor_copy(
                        ssb[32 * hh : 32 * hh + 1, :],
                        ps_out_prev[64:65, 512 * hh : 512 * (hh + 1)],
                    )
                rc32 = rcp.tile([33, 512], f32, name=f"rc32_{pr}_{jj}", tag="rc32")
                nc.vector.reciprocal_approx_fast(out=rc32[:], in_=ssb[:])
                rc = rcp.tile([33, 512], f32r, name=f"rc_{pr}_{jj}", tag="rc")
                nc.vector.tensor_copy(rc[:], rc32[:])
                bc = psS.tile([128, 1024], f32, name=f"ps_bc_{pr}_{jj}", tag="s")
                for hh in range(2):
                    nc.tensor.matmul(
                        bc[0:64, 512 * hh : 512 * (hh + 1)],
                        consts_sb[32 * hh : 32 * hh + 1, 128:192],
                        rc[32 * hh : 32 * hh + 1, :],
                        start=True,
                        stop=True,
                        tile_position=(32 * hh, 0),
                    )
                bc_sb = bcp.tile([64, 1024], f32, name=f"bc_sb_{pr}_{jj}", tag="bc")
                nc.vector.tensor_copy(bc_sb[:], bc[0:64, :])
                for hh in range(2):
                    nc.vector.tensor_mul(
                        outt_sb[pr][64 * hh : 64 * hh + 64, qsj],
                        ps_out_prev[0:64, 512 * hh : 512 * (hh + 1)],
                        bc_sb[:, 512 * hh : 512 * (hh + 1)],
                    )

            for pr in range(2):
                pending_norm = None  # (pr, j, ps_out) awaiting lazy normalize
                pending_out = None   # q-block awaiting output projection (pr=1)
                j_order = range(QT) if pr == 0 else range(QT - 1, -1, -1)
                for j in j_order:
                    n_i = 4 * j + 4
                    qs = slice(j * 512, (j + 1) * 512)
                    ps_out = psO.tile(
                        [65, 1024], f32, name=f"ps_out_{pr}_{j}", tag="o"
                    )
                    prev_et = None
                    prev_i = -1
                    for i in range(n_i):
                        diag = i >= 4 * j
                        r = i - 4 * j
                        pss = psS.tile(
                            [128, 1024], f32, name=f"ps_s{pr}_{j}_{i}", tag="s"
                        )
                        if diag:
                            nw = 128 * (r + 1)
                            for hh in range(2):
                                nc.tensor.matmul(
                                    pss[:, 512 * hh : 512 * hh + nw],
                                    idbf_sb[:, 0:128],
                                    maskt_sb[:, r * 512 : r * 512 + nw],
                                    start=True,
                                    stop=False,
                                )
                        for hh in range(2):
                            hp = slice(64 * hh, 64 * hh + 64)
                            nc.tensor.matmul(
                                pss[:, 512 * hh : 512 * (hh + 1)],
                                kt_sb[pr][hp, i * 128 : (i + 1) * 128],
                                qt_sb[pr][hp, qs],
                                start=not diag,
                                stop=True,
                            )
                        et = etp.tile(
                            [128, 1024], et_dt, name=f"et{pr}_{j}_{i}", tag="et"
                        )
                        nc.scalar.activation(et[:], pss[:], Exp, scale=0.125)
                        if prev_et is not None:
                            for hh in range(2):
                                nc.tensor.matmul(
                                    ps_out[:, 512 * hh : 512 * (hh + 1)],
                                    v_sb[prev_i][:, (2 * pr + hh) * 65 : (2 * pr + hh + 1) * 65],
                                    prev_et[:, 512 * hh : 512 * (hh + 1)],
                                    start=(prev_i == 0),
                                    stop=(prev_i == n_i - 1),
                                )
                        prev_et, prev_i = et, i
                        if i == 1 and pending_norm is not None:
                            pn_j = pending_norm[1]
                            emit_normalize(*pending_norm)
                            pending_norm = None
                            if pr == 1:
                                pending_out = pn_j
                        if pending_out is not None and i == 2:
                            for m in range(4 * pending_out, 4 * pending_out + 4):
                                emit_outproj_mtile(m)
                            pending_out = None
                    for hh in range(2):
                        nc.tensor.matmul(
                            ps_out[:, 512 * hh : 512 * (hh + 1)],
                            v_sb[n_i - 1][:, (2 * pr + hh) * 65 : (2 * pr + hh + 1) * 65],
                            prev_et[:, 512 * hh : 512 * (hh + 1)],
                            start=(n_i - 1 == 0),
                            stop=True,
                        )
                    pending_norm = (pr, j, ps_out)
                emit_normalize(*pending_norm)
                if pr == 1:
                    if pending_out is not None:
                        for m in range(4 * pending_out, 4 * pending_out + 4):
                            emit_outproj_mtile(m)
                    # pr1 runs j descending, so the pass ends on j=0
                    for m in range(0, 4):
                        emit_outproj_mtile(m)

    nc.compile()
    return nc


def _get_program():
    if "nc" not in _PROG_CACHE:
        _PROG_CACHE["nc"] = _build_program()
    return _PROG_CACHE["nc"]


def _host_prep(query, key, value, mask, w_q, w_k, w_v, w_o):
    query = np.asarray(query, dtype=np.float32)
    key = np.asarray(key, dtype=np.float32)
    value = np.asarray(value, dtype=np.float32)
    w_q = np.asarray(w_q, dtype=np.float32)
    w_k = np.asarray(w_k, dtype=np.float32)
    w_v = np.asarray(w_v, dtype=np.float32)
    w_o = np.asarray(w_o, dtype=np.float32)
    m = np.asarray(mask).reshape(S, S).astype(bool)

    # The kernel's block-skip structure assumes the standard causal mask.
    expected = np.triu(np.ones((S, S), dtype=bool), k=1)
    if not np.array_equal(m, expected):
        raise NotImplementedError("kernel specialized for causal (triu, k=1) mask")

    # 4 canonical diagonal-straddle mask tiles: pattern r covers k-tile
    # 4j+r vs q-tile j; masked where (128r + row) > col.
    import ml_dtypes

    maskt = np.zeros((128, 2048), dtype=np.float32)
    rows = np.arange(128)[:, None]
    cols = np.arange(512)[None, :]
    for r in range(4):
        maskt[:, r * 512 : (r + 1) * 512] = np.where(
            (128 * r + rows) > cols, np.float32(-1e9), np.float32(0.0)
        )
    maskt = maskt.astype(ml_dtypes.bfloat16)
    idbf = np.zeros((128, 132), dtype=ml_dtypes.bfloat16)
    idbf[:, 0:128] = np.eye(128, dtype=ml_dtypes.bfloat16)
    idbf[:, 128:132] = ml_dtypes.bfloat16(1.0)

    consts = np.zeros((128, 193), dtype=np.float32)
    consts[:, 0:128] = np.eye(128, dtype=np.float32)
    consts[:, 128:193] = 1.0

    x_np = ml_dtypes.bfloat16 if X_BF16 else np.float32

    xt = {}
    for b in range(B):
        xt[("q", b)] = np.ascontiguousarray(query[b].T.astype(x_np))
        xt[("k", b)] = np.ascontiguousarray(key[b].T.astype(x_np))
        xt[("v", b)] = np.ascontiguousarray(value[b].T.astype(x_np))

    in_maps = []
    for c in range(N_CORES):
        b = c // 4
        hb = (c % 4) * HPC
        rs = slice(hb * D_K, (hb + HPC) * D_K)
        in_maps.append(
            {
                "xq": xt[("q", b)],
                "xk": xt[("k", b)],
                "xv": xt[("v", b)],
                "wq": np.ascontiguousarray(w_q[rs, :].T.astype(x_np)),
                "wk": np.ascontiguousarray(w_k[rs, :].T.astype(x_np)),
                "wv": np.ascontiguousarray(w_v[rs, :].T.astype(x_np)),
                "wo": np.ascontiguousarray(w_o[:, rs].T),
                "maskt": maskt,
                "idbf": idbf,
                "consts": consts,
            }
        )
    return in_maps


def kernel(query, key, value, mask, w_q, w_k, w_v, w_o):
    from concourse.bass_utils import run_bass_kernel_spmd

    in_maps = _host_prep(query, key, value, mask, w_q, w_k, w_v, w_o)
    nc = _get_program()
    res = run_bass_kernel_spmd(nc, in_maps, list(range(N_CORES)))
    out = np.zeros((B, S, D_MODEL), dtype=np.float32)
    for c in range(N_CORES):
        out[c // 4] += res.results[c]["y"].astype(np.float32)
    return out



# revision 13
# speedup vs baseline: 1.3252x; 1.1249x over previous
"""Multi-head causal attention (B=2, S=2048, D=1024, H=16) on 8 TRN2 NeuronCores.

Sharding: batch*head parallel. Core c handles batch b = c//4 and the 4
heads h in [4*(c%4), 4*(c%4)+4). Each core computes its heads' Q/K/V
projections (column-parallel), causal softmax attention, and its partial
row-parallel output projection; the host sums the 4 partial outputs per
batch (the AllReduce of row-parallel tensor parallelism).

On-device layout: everything is kept "transposed" (feature-major) so
every matmul contracts along the partition dimension:
  scoresT[k,q] = K Q^T      (per head, 128-row k-tiles x 512-col q-tiles)
  P^T = exp(scoresT/8)      (diagonal blocks column-restricted; the
                             straddling 128x128 triangle is zeroed by a
                             DVE multiply with a 0/1 triangle tile)
  outT[d,q]   = sum_k V[k,d] P^T[k,q]   (PSUM-accumulated over k-tiles)
  sums[q]     = sum_k P^T[k,q]          (ones-vector matmul, col-packed)
  y[q,e]     += sum_hd outT_norm[hd,q] * w_oT[hd,e]
Softmax skips the max-subtraction: scores ~ N(0,1), so exp never
overflows fp32; fully-masked columns are simply never computed.

x / q,k,v weights / attention / outproj all run in bf16 (measured
~3.5e-3 max rel err vs the f32 reference, tolerance 2e-2). x is
host-pre-tiled to [128, 8*2048] (feature-major e-tiles side by side) and
held resident in SBUF, so Q/K/V projections all read the same resident
tiles (no strided V re-load). Constant tiles (ones / 0-1 triangle) are
generated on the idle GPSIMD engine (iota / affine_select) instead of
DMAed. Input DMAs are split ~0.5MB per queue and issued from both the
SP and Activation hardware DGE sequencers to halve issue serialization.

The PE HAM clock gate starts at half clock and only reaches full clock
after ~3.4us of gapless PE activity; a dummy warm-up pack (results never
read) runs while the x DMAs stream.
"""

import numpy as np

D_MODEL = 1024
N_HEADS = 16
D_K = 64
B, S = 2, 2048
N_CORES = 8
HPC = 4            # heads per core
KT = S // 128      # 16 k-tiles
QT = S // 512      # 4 q-tiles
ET = D_MODEL // 128  # 8 e-tiles (contraction tiles for projections)

_PROG_CACHE = {}


def _build_program():
    import concourse.bacc as bacc_mod
    import concourse.mybir as mybir
    import concourse.tile as tile

    f32 = mybir.dt.float32
    f32r = mybir.dt.float32r
    bf16 = mybir.dt.bfloat16
    Exp = mybir.ActivationFunctionType.Exp

    nc = bacc_mod.Bacc(
        "TRN2", target_bir_lowering=False, debug=False, num_devices=N_CORES
    )

    xq = nc.dram_tensor("xq", [128, ET * S], bf16, kind="ExternalInput").ap()
    xk = nc.dram_tensor("xk", [128, ET * S], bf16, kind="ExternalInput").ap()
    xv = nc.dram_tensor("xv", [128, ET * S], bf16, kind="ExternalInput").ap()
    wq = nc.dram_tensor("wq", [128, ET * 256], bf16, kind="ExternalInput").ap()
    wk = nc.dram_tensor("wk", [128, ET * 256], bf16, kind="ExternalInput").ap()
    wv = nc.dram_tensor("wv", [128, ET * 256], bf16, kind="ExternalInput").ap()
    wo = nc.dram_tensor("wo", [256, D_MODEL], bf16, kind="ExternalInput").ap()
    y = nc.dram_tensor("y", [S, D_MODEL], bf16, kind="ExternalOutput").ap()

    with (
        tile.TileContext(nc) as tc,
        nc.allow_low_precision("bf16 attention"),
        tc.tile_pool(name="persist", bufs=1) as pp,
    ):
        # ---- persistent SBUF tiles ----
        def persist(shape, dtype, name):
            return pp.tile(shape, dtype, name=name, tag=name)

        xq_sb = persist([128, ET * S], bf16, "xq_sb")
        xk_sb = persist([128, ET * S], bf16, "xk_sb")
        xv_sb = persist([128, ET * S], bf16, "xv_sb")
        wq_sb = persist([128, ET * 256], bf16, "wq_sb")
        wk_sb = persist([128, ET * 256], bf16, "wk_sb")
        wv_sb = persist([128, ET * 256], bf16, "wv_sb")
        wo_sb = [persist([128, D_MODEL], bf16, f"wo_sb{p}") for p in range(2)]
        gen_sb = persist([128, 512], bf16, "gen_sb")     # all-ones (iota)
        tri_sb = persist([128, 256], bf16, "tri_sb")     # 0/1 upper-tri x2
        onesf_sb = persist([128, 64], f32r, "onesf_sb")  # f32 ones (bc lhsT)
        qt_sb = [persist([128, S], bf16, f"qt_sb{p}") for p in range(2)]
        kt_sb = [persist([128, S], bf16, f"kt_sb{p}") for p in range(2)]
        v_sb = [persist([128, 260], bf16, f"v_sb{i}") for i in range(KT)]
        outt_sb = [persist([128, S], bf16, f"outt_sb{p}") for p in range(2)]

        # ---- GPSIMD-generated constants (no DMA dependencies) ----
        nc.gpsimd.iota(
            gen_sb[:], pattern=[[0, 512]], base=1, channel_multiplier=0,
            allow_small_or_imprecise_dtypes=True,
        )
        nc.vector.tensor_copy(onesf_sb[:], gen_sb[:, 0:64])
        # tri[p, a*128 + c] = 1.0 if p <= c else 0.0  (keep where col-row >= 0)
        nc.gpsimd.affine_select(
            tri_sb[:].rearrange("p (a c) -> p a c", a=2),
            gen_sb[:, 0:256].rearrange("p (a c) -> p a c", a=2),
            pattern=[[0, 2], [1, 128]],
            compare_op=mybir.AluOpType.is_ge,
            fill=0.0,
            base=0,
            channel_multiplier=-1,
        )

        # ---- input DMAs (split ~0.5MB/queue, dual-engine issue) ----
        # SP and Activation each drive their own 16 hardware DGE queues;
        # alternating the issuing engine halves the serial issue ramp.
        _eng = [nc.sync, nc.scalar]
        _n_dma = [0]

        def dma(out, in_):
            e = _eng[_n_dma[0] % 2]
            _n_dma[0] += 1
            e.dma_start(out=out, in_=in_)

        for w_dram, w_tile in ((wq, wq_sb), (wk, wk_sb)):
            for h in range(2):
                cs = slice(h * ET * 128, (h + 1) * ET * 128)
                dma(w_tile[:, cs], w_dram[:, cs])
        for t in range(ET):
            cs = slice(t * S, (t + 1) * S)
            dma(xq_sb[:, cs], xq[:, cs])
        for t in range(ET):
            cs = slice(t * S, (t + 1) * S)
            dma(xk_sb[:, cs], xk[:, cs])
        # xv in k-column chunks so early k-tiles of V can project first
        for kc in range(4):
            for eh in range(2):
                def v3(ap):
                    return ap.rearrange("p (t k) -> p t k", t=ET)[
                        :, eh * 4 : (eh + 1) * 4, kc * 512 : (kc + 1) * 512
                    ]
                dma(v3(xv_sb[:]), v3(xv))
        for h in range(2):
            cs = slice(h * ET * 128, (h + 1) * ET * 128)
            dma(wv_sb[:, cs], wv[:, cs])
        for p in range(2):
            dma(wo_sb[p][:], wo[p * 128 : (p + 1) * 128, :])

        # ---- PE warm-up ----
        # Dense dummy matmuls (results never read) force the HAM clock
        # gate to full speed while the x stream lands.
        def emit_warm_pack(pool, count, tag="warm", name="warm"):
            wt = pool.tile([128, 512], f32, name=name, tag=tag)
            for w in range(count):
                nc.tensor.matmul(
                    wt[:],
                    gen_sb[:, 0:128],
                    gen_sb[:, 0:512],
                    start=True,
                    stop=True,
                )

        with tc.tile_pool(name="psW", bufs=1, space="PSUM") as psW:
            emit_warm_pack(psW, 24, name="warm_start")

        # ---- phase B: projections (from resident x tiles) ----
        psA_ctx = tc.tile_pool(name="psA", bufs=1, space="PSUM")
        psA = psA_ctx.__enter__()
        for ti, (x_sb, w_tile, dst) in enumerate(
            ((xq_sb, wq_sb, qt_sb), (xk_sb, wk_sb, kt_sb))
        ):
            ps = [
                psA.tile([128, S], f32, name=f"ps_p{ti}_{m}", tag=f"proj{m}", bufs=1)
                for m in range(2)
            ]
            for e in range(ET):
                xe = x_sb[:, e * S : (e + 1) * S]
                for m in range(2):
                    lhsT = w_tile[:, e * 256 + m * 128 : e * 256 + (m + 1) * 128]
                    for n in range(QT):
                        nc.tensor.matmul(
                            ps[m][:, n * 512 : (n + 1) * 512],
                            lhsT,
                            xe[:, n * 512 : (n + 1) * 512],
                            start=(e == 0),
                            stop=(e == ET - 1),
                        )
            for m in range(2):
                nc.vector.tensor_copy(dst[m][:], ps[m][:])

        psA_ctx.__exit__(None, None, None)
        psV_ctx = tc.tile_pool(name="psV", bufs=2, space="PSUM")
        psV = psV_ctx.__enter__()
        for i in range(KT):
            psv = psV.tile([128, 256], f32, name=f"psv_{i}", tag="v")
            for e in range(ET):
                nc.tensor.matmul(
                    psv[:],
                    xv_sb[:, e * S + i * 128 : e * S + (i + 1) * 128],
                    wv_sb[:, e * 256 : (e + 1) * 256],
                    start=(e == 0),
                    stop=(e == ET - 1),
                )
            nc.vector.tensor_copy(
                v_sb[i][:].rearrange("p (h c) -> p h c", c=65)[:, :, 0:64],
                psv[:].rearrange("p (h d) -> p h d", d=64),
            )
            nc.vector.tensor_copy(
                v_sb[i][:].rearrange("p (h c) -> p h c", c=65)[:, :, 64:65],
                gen_sb[:, 0:4].rearrange("p (h c) -> p h c", c=1),
            )
        psV_ctx.__exit__(None, None, None)

        # ---- phase C+D: attention with interleaved output projection ----
        # One head-pair per pass (pr = 0, 1). Per (pr, j): score tiles are
        # [128, 1024] head-pair PSUM tiles; ONE exp per round (column-
        # restricted on diagonal blocks). attnV accumulates into a
        # [65, 1024] pair tile (row 64 = sum of exp via the ones column of
        # v_sb). Normalization of q-block j-1 is emitted lazily inside
        # block j; a staging copy releases its ps_out PSUM early so the
        # next block's attnV is not blocked. The output projection of
        # block j-1 runs as dense filler inside the pr=1 pass.
        with (
            tc.tile_pool(name="psS", bufs=3, space="PSUM") as psS,
            tc.tile_pool(name="psO", bufs=1, space="PSUM") as psO,
            tc.tile_pool(name="et", bufs=6) as etp,
            tc.tile_pool(name="bcsb", bufs=3) as bcp,
            tc.tile_pool(name="rcsb", bufs=3) as rcp,
            tc.tile_pool(name="ysb", bufs=3) as ysbp,
        ):
            tri3 = tri_sb[:].rearrange("p (a c) -> p a c", a=2)

            def et3(t):
                return t[:].rearrange("p (h q) -> p h q", h=2)

            def emit_outproj_mtile(m):
                psy = psS.tile([128, 1024], f32, name=f"psy_{m}", tag="s")
                for p in range(2):
                    for n in range(2):
                        nc.tensor.matmul(
                            psy[:, n * 512 : (n + 1) * 512],
                            outt_sb[p][:, m * 128 : (m + 1) * 128],
                            wo_sb[p][:, n * 512 : (n + 1) * 512],
                            start=(p == 0),
                            stop=(p == 1),
                        )
                y_sb = ysbp.tile([128, 1024], bf16, name=f"y_sb_{m}", tag="ysb")
                nc.vector.tensor_copy(y_sb[:], psy[:])
                nc.sync.dma_start(out=y[m * 128 : (m + 1) * 128, :], in_=y_sb[:])

            def emit_normalize(pr, jj, ps_out_prev):
                qsj = slice(jj * 512, (jj + 1) * 512)
                ssb = rcp.tile([33, 512], f32, name=f"ssb_{pr}_{jj}", tag="ssb")
                for hh in range(2):
                    nc.vector.tensor_copy(
                        ssb[32 * hh : 32 * hh + 1, :],
                        ps_out_prev[64:65, 512 * hh : 512 * (hh + 1)],
                    )
                # staging copy releases ps_out for the next block's attnV
                stg = rcp.tile([64, 1024], f32, name=f"stg_{pr}_{jj}", tag="stg")
                nc.vector.tensor_copy(stg[:], ps_out_prev[0:64, :])
                rc32 = rcp.tile([33, 512], f32, name=f"rc32_{pr}_{jj}", tag="rc32")
                nc.vector.reciprocal_approx_fast(out=rc32[:], in_=ssb[:])
                rc = rcp.tile([33, 512], f32r, name=f"rc_{pr}_{jj}", tag="rc")
                nc.vector.tensor_copy(rc[:], rc32[:])
                bc = psS.tile([128, 1024], f32, name=f"ps_bc_{pr}_{jj}", tag="s")
                for hh in range(2):
                    nc.tensor.matmul(
                        bc[0:64, 512 * hh : 512 * (hh + 1)],
                        onesf_sb[32 * hh : 32 * hh + 1, 0:64],
                        rc[32 * hh : 32 * hh + 1, :],
                        start=True,
                        stop=True,
                        tile_position=(32 * hh, 0),
                    )
                bc_sb = bcp.tile([64, 1024], f32, name=f"bc_sb_{pr}_{jj}", tag="bc")
                nc.vector.tensor_copy(bc_sb[:], bc[0:64, :])
                for hh in range(2):
                    nc.vector.tensor_mul(
                        outt_sb[pr][64 * hh : 64 * hh + 64, qsj],
                        stg[:, 512 * hh : 512 * (hh + 1)],
                        bc_sb[:, 512 * hh : 512 * (hh + 1)],
                    )

            for pr in range(2):
                pending_norm = None  # (pr, j, ps_out) awaiting lazy normalize
                pending_out = None   # q-block awaiting output projection (pr=1)
                j_order = range(QT) if pr == 0 else range(QT - 1, -1, -1)
                for j in j_order:
                    n_i = 4 * j + 4
                    ps_out = psO.tile(
                        [65, 1024], f32, name=f"ps_out_{pr}_{j}", tag="o"
                    )
                    prev_et = None
                    prev_i = -1
                    prev_lo = 0
                    for i in range(n_i):
                        diag = i >= 4 * j
                        r = i - 4 * j
                        lo = 128 * r if diag else 0
                        pss = psS.tile(
                            [128, 1024], f32, name=f"ps_s{pr}_{j}_{i}", tag="s"
                        )
                        for hh in range(2):
                            hp = slice(64 * hh, 64 * hh + 64)
                            nc.tensor.matmul(
                                pss[:, 512 * hh + lo : 512 * (hh + 1)],
                                kt_sb[pr][hp, i * 128 : (i + 1) * 128],
                                qt_sb[pr][hp, j * 512 + lo : (j + 1) * 512],
                                start=True,
                                stop=True,
                            )
                        et = etp.tile(
                            [128, 1024], bf16, name=f"et{pr}_{j}_{i}", tag="et"
                        )
                        if lo:
                            nc.scalar.activation(
                                et3(et)[:, :, lo:], et3(pss)[:, :, lo:],
                                Exp, scale=0.125,
                            )
                        else:
                            nc.scalar.activation(et[:], pss[:], Exp, scale=0.125)
                        if diag:
                            # zero the masked triangle of the straddling block
                            nc.vector.tensor_mul(
                                et3(et)[:, :, lo : lo + 128],
                                et3(et)[:, :, lo : lo + 128],
                                tri3,
                            )
                        if prev_et is not None:
                            for hh in range(2):
                                nc.tensor.matmul(
                                    ps_out[:, 512 * hh + prev_lo : 512 * (hh + 1)],
                                    v_sb[prev_i][:, (2 * pr + hh) * 65 : (2 * pr + hh + 1) * 65],
                                    prev_et[:, 512 * hh + prev_lo : 512 * (hh + 1)],
                                    start=(prev_i == 0),
                                    stop=(prev_i == n_i - 1),
                                    skip_group_check=True,
                                )
                        prev_et, prev_i, prev_lo = et, i, lo
                        if i == 1 and pending_norm is not None:
                            pn_j = pending_norm[1]
                            emit_normalize(*pending_norm)
                            pending_norm = None
                            if pr == 1:
                                pending_out = pn_j
                        if pending_out is not None and i == 2:
                            for m in range(4 * pending_out, 4 * pending_out + 4):
                                emit_outproj_mtile(m)
                            pending_out = None
                    for hh in range(2):
                        nc.tensor.matmul(
                            ps_out[:, 512 * hh + prev_lo : 512 * (hh + 1)],
                            v_sb[n_i - 1][:, (2 * pr + hh) * 65 : (2 * pr + hh + 1) * 65],
                            prev_et[:, 512 * hh + prev_lo : 512 * (hh + 1)],
                            start=(n_i - 1 == 0),
                            stop=True,
                            skip_group_check=True,
                        )
                    pending_norm = (pr, j, ps_out)
                emit_normalize(*pending_norm)
                if pr == 1:
                    if pending_out is not None:
                        for m in range(4 * pending_out, 4 * pending_out + 4):
                            emit_outproj_mtile(m)
                    # pr1 runs j descending, so the pass ends on j=0
                    for m in range(0, 4):
                        emit_outproj_mtile(m)

    nc.compile()
    return nc


def _get_program():
    if "nc" not in _PROG_CACHE:
        _PROG_CACHE["nc"] = _build_program()
    return _PROG_CACHE["nc"]


def _host_prep(query, key, value, mask, w_q, w_k, w_v, w_o):
    import ml_dtypes

    bf = ml_dtypes.bfloat16
    query = np.asarray(query, dtype=np.float32)
    key = np.asarray(key, dtype=np.float32)
    value = np.asarray(value, dtype=np.float32)
    w_q = np.asarray(w_q, dtype=np.float32)
    w_k = np.asarray(w_k, dtype=np.float32)
    w_v = np.asarray(w_v, dtype=np.float32)
    w_o = np.asarray(w_o, dtype=np.float32)
    m = np.asarray(mask).reshape(S, S).astype(bool)

    # The kernel's block-skip structure assumes the standard causal mask.
    expected = np.triu(np.ones((S, S), dtype=bool), k=1)
    if not np.array_equal(m, expected):
        raise NotImplementedError("kernel specialized for causal (triu, k=1) mask")

    def tile_x(xT):  # [1024, 2048] -> [128, 8*2048] (e-tiles side by side)
        return np.ascontiguousarray(
            xT.reshape(ET, 128, S).transpose(1, 0, 2).reshape(128, ET * S).astype(bf)
        )

    def tile_w(w_rows):  # [256, 1024] slice -> [128, 8*256]
        t = w_rows.T.reshape(ET, 128, 256).transpose(1, 0, 2).reshape(128, ET * 256)
        return np.ascontiguousarray(t.astype(bf))

    xt = {}
    for b in range(B):
        xt[("q", b)] = tile_x(query[b].T)
        xt[("k", b)] = tile_x(key[b].T)
        xt[("v", b)] = tile_x(value[b].T)

    in_maps = []
    for c in range(N_CORES):
        b = c // 4
        hb = (c % 4) * HPC
        rs = slice(hb * D_K, (hb + HPC) * D_K)
        in_maps.append(
            {
                "xq": xt[("q", b)],
                "xk": xt[("k", b)],
                "xv": xt[("v", b)],
                "wq": tile_w(w_q[rs, :]),
                "wk": tile_w(w_k[rs, :]),
                "wv": tile_w(w_v[rs, :]),
                "wo": np.ascontiguousarray(w_o[:, rs].T.astype(bf)),
            }
        )
    return in_maps


def kernel(query, key, value, mask, w_q, w_k, w_v, w_o):
    from concourse.bass_utils import run_bass_kernel_spmd

    in_maps = _host_prep(query, key, value, mask, w_q, w_k, w_v, w_o)
    nc = _get_program()
    res = run_bass_kernel_spmd(nc, in_maps, list(range(N_CORES)))
    out = np.zeros((B, S, D_MODEL), dtype=np.float32)
    for c in range(N_CORES):
        out[c // 4] += res.results[c]["y"].astype(np.float32)
    return out


# revision 23
# speedup vs baseline: 1.3582x; 1.0249x over previous
"""Multi-head causal attention (B=2, S=2048, D=1024, H=16) on 8 TRN2 NeuronCores.

Sharding: batch*head parallel. Core c handles batch b = c//4 and the 4
heads h in [4*(c%4), 4*(c%4)+4). Each core computes its heads' Q/K/V
projections (column-parallel), causal softmax attention, and its partial
row-parallel output projection; the host sums the 4 partial outputs per
batch (the AllReduce of row-parallel tensor parallelism).

On-device layout: everything is kept "transposed" (feature-major) so
every matmul contracts along the partition dimension:
  scoresT[k,q] = K Q^T      (per head, 128-row k-tiles x 512-col q-tiles)
  P^T = exp(scoresT/8)      (diagonal blocks column-restricted; the
                             straddling 128x128 triangle is zeroed by a
                             DVE multiply with a 0/1 triangle tile)
  outT[d,q]   = sum_k V[k,d] P^T[k,q]   (PSUM-accumulated over k-tiles)
  sums[q]     = sum_k P^T[k,q]          (ones-vector matmul, col-packed)
  y[q,e]     += sum_hd outT_norm[hd,q] * w_oT[hd,e]
Softmax skips the max-subtraction: scores ~ N(0,1), so exp never
overflows fp32; fully-masked columns are simply never computed.

x / q,k,v weights / attention / outproj all run in bf16 (measured
~3.5e-3 max rel err vs the f32 reference, tolerance 2e-2). x is
host-pre-tiled to [128, 8*2048] (feature-major e-tiles side by side) and
held resident in SBUF, so Q/K/V projections all read the same resident
tiles (no strided V re-load). Constant tiles (ones / 0-1 triangle) are
generated on the idle GPSIMD engine (iota / affine_select) instead of
DMAed. Input DMAs are split ~0.5MB per queue and issued from both the
SP and Activation hardware DGE sequencers to halve issue serialization.

The PE HAM clock gate starts at half clock and only reaches full clock
after ~3.4us of gapless PE activity; a dummy warm-up pack (results never
read) runs while the x DMAs stream.
"""

import numpy as np

D_MODEL = 1024
N_HEADS = 16
D_K = 64
B, S = 2, 2048
N_CORES = 8
HPC = 4            # heads per core
KT = S // 128      # 16 k-tiles
QT = S // 512      # 4 q-tiles
ET = D_MODEL // 128  # 8 e-tiles (contraction tiles for projections)

_PROG_CACHE = {}


def _build_program():
    import concourse.bacc as bacc_mod
    import concourse.mybir as mybir
    import concourse.tile as tile

    f32 = mybir.dt.float32
    f32r = mybir.dt.float32r
    bf16 = mybir.dt.bfloat16
    Exp = mybir.ActivationFunctionType.Exp

    nc = bacc_mod.Bacc(
        "TRN2", target_bir_lowering=False, debug=False, num_devices=N_CORES
    )

    xq = nc.dram_tensor("xq", [128, ET * S], bf16, kind="ExternalInput").ap()
    xk = nc.dram_tensor("xk", [128, ET * S], bf16, kind="ExternalInput").ap()
    xv = nc.dram_tensor("xv", [128, ET * S], bf16, kind="ExternalInput").ap()
    wq = nc.dram_tensor("wq", [128, ET * 256], bf16, kind="ExternalInput").ap()
    wk = nc.dram_tensor("wk", [128, ET * 256], bf16, kind="ExternalInput").ap()
    wv = nc.dram_tensor("wv", [128, ET * 256], bf16, kind="ExternalInput").ap()
    wo = nc.dram_tensor("wo", [256, D_MODEL], bf16, kind="ExternalInput").ap()
    y = nc.dram_tensor("y", [S, D_MODEL], bf16, kind="ExternalOutput").ap()

    with (
        tile.TileContext(nc) as tc,
        nc.allow_low_precision("bf16 attention"),
        tc.tile_pool(name="persist", bufs=1) as pp,
    ):
        # ---- persistent SBUF tiles ----
        def persist(shape, dtype, name):
            return pp.tile(shape, dtype, name=name, tag=name)

        xq_sb = persist([128, ET * S], bf16, "xq_sb")
        xk_sb = persist([128, ET * S], bf16, "xk_sb")
        xv_sb = persist([128, ET * S], bf16, "xv_sb")
        wq_sb = persist([128, ET * 256], bf16, "wq_sb")
        wk_sb = persist([128, ET * 256], bf16, "wk_sb")
        wv_sb = persist([128, ET * 256], bf16, "wv_sb")
        wo_sb = [persist([128, D_MODEL], bf16, f"wo_sb{p}") for p in range(2)]
        gen_sb = persist([128, 512], bf16, "gen_sb")     # all-ones (iota)
        tri_sb = persist([128, 256], bf16, "tri_sb")     # 0/1 upper-tri x2
        onesf_sb = persist([128, 64], f32r, "onesf_sb")  # ones (bc lhsT)
        qt_sb = [persist([128, S], bf16, f"qt_sb{p}") for p in range(2)]
        kt_sb = [persist([128, S], bf16, f"kt_sb{p}") for p in range(2)]
        v_sb = [persist([128, 260], bf16, f"v_sb{i}") for i in range(KT)]
        outt_sb = [persist([128, S], bf16, f"outt_sb{p}") for p in range(2)]

        # ---- GPSIMD-generated constants (no DMA dependencies) ----
        nc.gpsimd.iota(
            gen_sb[:], pattern=[[0, 512]], base=1, channel_multiplier=0,
            allow_small_or_imprecise_dtypes=True,
        )
        nc.vector.tensor_copy(onesf_sb[:], gen_sb[:, 0:64])
        # tri[p, a*128 + c] = 1.0 if p <= c else 0.0  (keep where col-row >= 0)
        nc.gpsimd.affine_select(
            tri_sb[:].rearrange("p (a c) -> p a c", a=2),
            gen_sb[:, 0:256].rearrange("p (a c) -> p a c", a=2),
            pattern=[[0, 2], [1, 128]],
            compare_op=mybir.AluOpType.is_ge,
            fill=0.0,
            base=0,
            channel_multiplier=-1,
        )

        # ---- input DMAs (split ~0.5MB/queue, dual-engine issue) ----
        # SP and Activation each drive their own 16 hardware DGE queues;
        # alternating the issuing engine halves the serial issue ramp.
        _eng = [nc.sync, nc.scalar]
        _n_dma = [0]

        def dma(out, in_):
            e = _eng[_n_dma[0] % 2]
            _n_dma[0] += 1
            e.dma_start(out=out, in_=in_)

        for w_dram, w_tile in ((wq, wq_sb), (wk, wk_sb)):
            for h in range(2):
                cs = slice(h * ET * 128, (h + 1) * ET * 128)
                dma(w_tile[:, cs], w_dram[:, cs])
        for t in range(ET):
            cs = slice(t * S, (t + 1) * S)
            dma(xq_sb[:, cs], xq[:, cs])
        for t in range(ET):
            cs = slice(t * S, (t + 1) * S)
            dma(xk_sb[:, cs], xk[:, cs])
        # xv in k-column chunks so early k-tiles of V can project first
        for kc in range(4):
            for eh in range(2):
                def v3(ap):
                    return ap.rearrange("p (t k) -> p t k", t=ET)[
                        :, eh * 4 : (eh + 1) * 4, kc * 512 : (kc + 1) * 512
                    ]
                dma(v3(xv_sb[:]), v3(xv))
        for h in range(2):
            cs = slice(h * ET * 128, (h + 1) * ET * 128)
            dma(wv_sb[:, cs], wv[:, cs])
        for p in range(2):
            dma(wo_sb[p][:], wo[p * 128 : (p + 1) * 128, :])

        # ---- PE warm-up ----
        # Dense dummy matmuls (results never read) force the HAM clock
        # gate to full speed while the x stream lands.
        def emit_warm_pack(pool, count, tag="warm", name="warm"):
            wt = pool.tile([128, 512], f32, name=name, tag=tag)
            for w in range(count):
                nc.tensor.matmul(
                    wt[:],
                    gen_sb[:, 0:128],
                    gen_sb[:, 0:512],
                    start=True,
                    stop=True,
                )

        with tc.tile_pool(name="psW", bufs=1, space="PSUM") as psW:
            emit_warm_pack(psW, 24, name="warm_start")

        # ---- phase B: projections (from resident x tiles) ----
        psA_ctx = tc.tile_pool(name="psA", bufs=1, space="PSUM")
        psA = psA_ctx.__enter__()
        for ti, (x_sb, w_tile, dst) in enumerate(
            ((xq_sb, wq_sb, qt_sb), (xk_sb, wk_sb, kt_sb))
        ):
            ps = [
                psA.tile([128, S], f32, name=f"ps_p{ti}_{m}", tag=f"proj{m}", bufs=1)
                for m in range(2)
            ]
            for e in range(ET):
                xe = x_sb[:, e * S : (e + 1) * S]
                for m in range(2):
                    lhsT = w_tile[:, e * 256 + m * 128 : e * 256 + (m + 1) * 128]
                    for n in range(QT):
                        nc.tensor.matmul(
                            ps[m][:, n * 512 : (n + 1) * 512],
                            lhsT,
                            xe[:, n * 512 : (n + 1) * 512],
                            start=(e == 0),
                            stop=(e == ET - 1),
                        )
            for m in range(2):
                nc.vector.tensor_copy(dst[m][:], ps[m][:])

        psA_ctx.__exit__(None, None, None)
        psV_ctx = tc.tile_pool(name="psV", bufs=2, space="PSUM")
        psV = psV_ctx.__enter__()
        for i in range(KT):
            psv = psV.tile([128, 256], f32, name=f"psv_{i}", tag="v")
            for e in range(ET):
                nc.tensor.matmul(
                    psv[:],
                    xv_sb[:, e * S + i * 128 : e * S + (i + 1) * 128],
                    wv_sb[:, e * 256 : (e + 1) * 256],
                    start=(e == 0),
                    stop=(e == ET - 1),
                )
            nc.vector.tensor_copy(
                v_sb[i][:].rearrange("p (h c) -> p h c", c=65)[:, :, 0:64],
                psv[:].rearrange("p (h d) -> p h d", d=64),
            )
            nc.vector.tensor_copy(
                v_sb[i][:].rearrange("p (h c) -> p h c", c=65)[:, :, 64:65],
                gen_sb[:, 0:4].rearrange("p (h c) -> p h c", c=1),
            )
        psV_ctx.__exit__(None, None, None)

        # ---- phase C+D: attention with interleaved output projection ----
        # One head-pair per pass (pr = 0, 1). Per (pr, j): score tiles are
        # [128, 1024] head-pair PSUM tiles; ONE exp per round (column-
        # restricted on diagonal blocks). attnV accumulates into a
        # [65, 1024] pair tile (row 64 = sum of exp via the ones column of
        # v_sb). Normalization of q-block j-1 is emitted lazily inside
        # block j; a staging copy releases its ps_out PSUM early so the
        # next block's attnV is not blocked. The output projection of
        # block j-1 runs as dense filler inside the pr=1 pass.
        with (
            tc.tile_pool(name="psS", bufs=2, space="PSUM") as psS,
            tc.tile_pool(name="psO", bufs=1, space="PSUM") as psO,
            tc.tile_pool(name="et", bufs=6) as etp,
            tc.tile_pool(name="bcsb", bufs=3) as bcp,
            tc.tile_pool(name="rcsb", bufs=3) as rcp,
            tc.tile_pool(name="ysb", bufs=3) as ysbp,
        ):
            tri3 = tri_sb[:].rearrange("p (a c) -> p a c", a=2)

            def et3(t):
                return t[:].rearrange("p (h q) -> p h q", h=2)

            def emit_outproj_mtile(m):
                psy = psS.tile([128, 1024], f32, name=f"psy_{m}", tag="s")
                for p in range(2):
                    for n in range(2):
                        nc.tensor.matmul(
                            psy[:, n * 512 : (n + 1) * 512],
                            outt_sb[p][:, m * 128 : (m + 1) * 128],
                            wo_sb[p][:, n * 512 : (n + 1) * 512],
                            start=(p == 0),
                            stop=(p == 1),
                        )
                y_sb = ysbp.tile([128, 1024], bf16, name=f"y_sb_{m}", tag="ysb")
                nc.vector.tensor_copy(y_sb[:], psy[:])
                for n in range(2):
                    nc.sync.dma_start(
                        out=y[m * 128 : (m + 1) * 128, n * 512 : (n + 1) * 512],
                        in_=y_sb[:, n * 512 : (n + 1) * 512],
                    )

            def emit_normalize(pr, jj, ps_out_prev):
                qsj = slice(jj * 512, (jj + 1) * 512)
                ssb = rcp.tile([33, 512], f32, name=f"ssb_{pr}_{jj}", tag="ssb")
                for hh in range(2):
                    nc.vector.tensor_copy(
                        ssb[32 * hh : 32 * hh + 1, :],
                        ps_out_prev[64:65, 512 * hh : 512 * (hh + 1)],
                    )
                # staging copy releases ps_out for the next block's attnV
                stg = rcp.tile([64, 1024], f32, name=f"stg_{pr}_{jj}", tag="stg")
                nc.vector.tensor_copy(stg[:], ps_out_prev[0:64, :])
                rc32 = rcp.tile([33, 512], f32, name=f"rc32_{pr}_{jj}", tag="rc32")
                nc.vector.reciprocal_approx_fast(out=rc32[:], in_=ssb[:])
                rc = rcp.tile([33, 512], f32r, name=f"rc_{pr}_{jj}", tag="rc")
                nc.vector.tensor_copy(rc[:], rc32[:])
                bc = psS.tile(
                    [64, 1024], f32, name=f"ps_bc_{pr}_{jj}", tag="bc", bufs=1
                )
                for hh in range(2):
                    nc.tensor.matmul(
                        bc[0:64, 512 * hh : 512 * (hh + 1)],
                        onesf_sb[32 * hh : 32 * hh + 1, 0:64],
                        rc[32 * hh : 32 * hh + 1, :],
                        start=True,
                        stop=True,
                        tile_position=(32 * hh, 0),
                    )
                bc_sb = bcp.tile([64, 1024], f32, name=f"bc_sb_{pr}_{jj}", tag="bc")
                nc.vector.tensor_copy(bc_sb[:], bc[:])
                for hh in range(2):
                    nc.vector.tensor_mul(
                        outt_sb[pr][64 * hh : 64 * hh + 64, qsj],
                        stg[:, 512 * hh : 512 * (hh + 1)],
                        bc_sb[:, 512 * hh : 512 * (hh + 1)],
                    )

            pending_norms = []  # (pr, j, ps_out) awaiting lazy normalize
            ready_out = []      # j's with both norms done, awaiting outproj
            for pr in range(2):
                j_order = range(QT) if pr == 0 else range(QT - 1, -1, -1)
                for j in j_order:
                    n_i = 4 * j + 4
                    ps_out = psO.tile(
                        [65, 1024], f32, name=f"ps_out_{pr}_{j}", tag="o"
                    )
                    prev_et = None
                    prev_i = -1
                    prev_lo = 0
                    for i in range(n_i):
                        diag = i >= 4 * j
                        r = i - 4 * j
                        lo = 128 * r if diag else 0
                        pss = psS.tile(
                            [128, 1024], f32, name=f"ps_s{pr}_{j}_{i}", tag="s"
                        )
                        for hh in range(2):
                            hp = slice(64 * hh, 64 * hh + 64)
                            nc.tensor.matmul(
                                pss[:, 512 * hh + lo : 512 * (hh + 1)],
                                kt_sb[pr][hp, i * 128 : (i + 1) * 128],
                                qt_sb[pr][hp, j * 512 + lo : (j + 1) * 512],
                                start=True,
                                stop=True,
                            )
                        et = etp.tile(
                            [128, 1024], bf16, name=f"et{pr}_{j}_{i}", tag="et"
                        )
                        if lo:
                            nc.scalar.activation(
                                et3(et)[:, :, lo:], et3(pss)[:, :, lo:],
                                Exp, scale=0.125,
                            )
                        else:
                            nc.scalar.activation(et[:], pss[:], Exp, scale=0.125)
                        if diag:
                            # zero the masked triangle of the straddling block
                            nc.vector.tensor_mul(
                                et3(et)[:, :, lo : lo + 128],
                                et3(et)[:, :, lo : lo + 128],
                                tri3,
                            )
                        if i == 1:
                            # lazy normalizes of previous block(s): emitted
                            # BEFORE this block's first attnV so their ps_out
                            # reads precede its overwrite in program order
                            # (psO has a single buffer).
                            for pn in pending_norms:
                                emit_normalize(*pn)
                                if pn[0] == 1:
                                    ready_out.append(pn[1])
                            pending_norms.clear()
                        if prev_et is not None:
                            for hh in range(2):
                                nc.tensor.matmul(
                                    ps_out[:, 512 * hh + prev_lo : 512 * (hh + 1)],
                                    v_sb[prev_i][:, (2 * pr + hh) * 65 : (2 * pr + hh + 1) * 65],
                                    prev_et[:, 512 * hh + prev_lo : 512 * (hh + 1)],
                                    start=(prev_i == 0),
                                    stop=(prev_i == n_i - 1),
                                    skip_group_check=True,
                                )
                        prev_et, prev_i, prev_lo = et, i, lo
                        if i == 2 and ready_out:
                            for jj in ready_out:
                                for m in range(4 * jj, 4 * jj + 4):
                                    emit_outproj_mtile(m)
                            ready_out.clear()
                    for hh in range(2):
                        nc.tensor.matmul(
                            ps_out[:, 512 * hh + prev_lo : 512 * (hh + 1)],
                            v_sb[n_i - 1][:, (2 * pr + hh) * 65 : (2 * pr + hh + 1) * 65],
                            prev_et[:, 512 * hh + prev_lo : 512 * (hh + 1)],
                            start=(n_i - 1 == 0),
                            stop=True,
                            skip_group_check=True,
                        )
                    pending_norms.append((pr, j, ps_out))
            # drain: pr1 ends on j=0
            for pn in pending_norms:
                emit_normalize(*pn)
                if pn[0] == 1:
                    ready_out.append(pn[1])
            pending_norms.clear()
            for jj in ready_out:
                for m in range(4 * jj, 4 * jj + 4):
                    emit_outproj_mtile(m)
            ready_out.clear()

    nc.compile()
    return nc


def _get_program():
    if "nc" not in _PROG_CACHE:
        _PROG_CACHE["nc"] = _build_program()
    return _PROG_CACHE["nc"]


def _host_prep(query, key, value, mask, w_q, w_k, w_v, w_o):
    import ml_dtypes

    bf = ml_dtypes.bfloat16
    query = np.asarray(query, dtype=np.float32)
    key = np.asarray(key, dtype=np.float32)
    value = np.asarray(value, dtype=np.float32)
    w_q = np.asarray(w_q, dtype=np.float32)
    w_k = np.asarray(w_k, dtype=np.float32)
    w_v = np.asarray(w_v, dtype=np.float32)
    w_o = np.asarray(w_o, dtype=np.float32)
    m = np.asarray(mask).reshape(S, S).astype(bool)

    # The kernel's block-skip structure assumes the standard causal mask.
    expected = np.triu(np.ones((S, S), dtype=bool), k=1)
    if not np.array_equal(m, expected):
        raise NotImplementedError("kernel specialized for causal (triu, k=1) mask")

    def tile_x(xT):  # [1024, 2048] -> [128, 8*2048] (e-tiles side by side)
        return np.ascontiguousarray(
            xT.reshape(ET, 128, S).transpose(1, 0, 2).reshape(128, ET * S).astype(bf)
        )

    def tile_w(w_rows):  # [256, 1024] slice -> [128, 8*256]
        t = w_rows.T.reshape(ET, 128, 256).transpose(1, 0, 2).reshape(128, ET * 256)
        return np.ascontiguousarray(t.astype(bf))

    xt = {}
    for b in range(B):
        xt[("q", b)] = tile_x(query[b].T)
        xt[("k", b)] = tile_x(key[b].T)
        xt[("v", b)] = tile_x(value[b].T)

    in_maps = []
    for c in range(N_CORES):
        b = c // 4
        hb = (c % 4) * HPC
        rs = slice(hb * D_K, (hb + HPC) * D_K)
        in_maps.append(
            {
                "xq": xt[("q", b)],
                "xk": xt[("k", b)],
                "xv": xt[("v", b)],
                "wq": tile_w(w_q[rs, :]),
                "wk": tile_w(w_k[rs, :]),
                "wv": tile_w(w_v[rs, :]),
                "wo": np.ascontiguousarray(w_o[:, rs].T.astype(bf)),
            }
        )
    return in_maps


def kernel(query, key, value, mask, w_q, w_k, w_v, w_o):
    from concourse.bass_utils import run_bass_kernel_spmd

    in_maps = _host_prep(query, key, value, mask, w_q, w_k, w_v, w_o)
    nc = _get_program()
    res = run_bass_kernel_spmd(nc, in_maps, list(range(N_CORES)))
    out = np.zeros((B, S, D_MODEL), dtype=np.float32)
    for c in range(N_CORES):
        out[c // 4] += res.results[c]["y"].astype(np.float32)
    return out
